# revision 11
# baseline (speedup 1.0000x reference)
"""Trainium2 Bass kernel for causal self-attention (GQA, RoPE, q/k-RMSNorm).

Sharding: tensor-parallel over heads across 8 cores.
  - core c owns q-heads [4c, 4c+4) and kv-head c//2 (each kv head serves 8 q heads)
  - x^T is built on-device (PE transposes of each core's column slice + AllGather)
  - attention is computed transposed (E^T = exp(K·Q^T)) so V in natural [S,D]
    layout is the matmul lhsT and y^T comes out in [D,T] layout directly
  - y^T is AllGathered (bf16), o_proj is column-sharded: core c computes
    out^T[:, 256c:256c+256]^T = Wo[256c:256c+256,:] @ y^T_full
  - head-dim rows of q/k are interleaved (d -> [0,64,1,65,...]) so the RoPE
    rotate-half becomes an adjacent-pair partition swap (one stream_shuffle)

Matmul dtypes: QKV + o_proj in bf16 (fp32 PSUM accum), attention in float32r.
"""

import sys

sys.path.insert(0, "/opt/trn_rl_repo")

from contextlib import ExitStack

import numpy as np

import bass_rust
import concourse.bass as bass
import concourse.mybir as mybir
from concourse import tile
from concourse.bass_utils import run_bass_kernel_spmd

F32 = mybir.dt.float32
F32R = mybir.dt.float32r
BF16 = mybir.dt.bfloat16

N_HEAD = 32
N_KV = 4
D = 128
C = 2048
T = 2048
NCORES = 8
HPC = N_HEAD // NCORES  # q heads per core = 4
THETA = 1000000.0
EPS = 1e-6
SCALE = 1.0 / np.sqrt(128.0)

NT = T // 512  # 4 T-chunks of 512
NK = C // 128  # 16 contraction tiles for qkv
NS = T // 128  # 16 S-blocks of 128

# stream_shuffle swaps within each 32-partition quadrant; adjacent-pair swap
SWAP_MASK = [i ^ 1 for i in range(32)]

_BF16_NP = None


def _bf16():
    global _BF16_NP
    if _BF16_NP is None:
        import ml_dtypes

        _BF16_NP = np.dtype(ml_dtypes.bfloat16)
    return _BF16_NP


def split_multiwaits(nc):
    """The walrus build in this container supports one sync-wait per
    instruction; hoist extra waits onto NOPs inserted before the offender."""
    ctr = 0
    for f in nc.m.functions:
        for bb in f.blocks:
            new_insts = []
            changed = False
            for inst in bb.instructions:
                si = inst.sync_info
                if si is not None and si.on_wait and len(si.on_wait) > 1:
                    waits = list(si.on_wait)
                    for w in waits[:-1]:
                        ctr += 1
                        nop = bass_rust.InstNoOp(
                            name=f"splitw-{ctr}", ins=[], outs=[]
                        )
                        nop.engine = inst.engine
                        nop.sync_info = bass_rust.SyncInfo(
                            on_wait=[w], on_update=[]
                        )
                        new_insts.append(nop)
                    inst.sync_info = bass_rust.SyncInfo(
                        on_wait=[waits[-1]], on_update=list(si.on_update or [])
                    )
                    changed = True
                new_insts.append(inst)
            if changed:
                bb.instructions = new_insts


def build_program():
    nc = bass.Bass("TRN2", target_bir_lowering=False, debug=False, num_devices=NCORES)

    xs = nc.declare_dram_parameter("xs", [T, 256], F32, isOutput=False)
    wq = nc.declare_dram_parameter("wq", [128, HPC * NK * 128], BF16, isOutput=False)
    wk = nc.declare_dram_parameter("wk", [128, NK * 128], BF16, isOutput=False)
    wv = nc.declare_dram_parameter("wv", [128, NK * 128], BF16, isOutput=False)
    wo = nc.declare_dram_parameter("wo", [128, 64 * 128], BF16, isOutput=False)
    cq = nc.declare_dram_parameter("cq", [128, T], F32, isOutput=False)
    sq = nc.declare_dram_parameter("sq", [128, T], F32, isOutput=False)
    ck = nc.declare_dram_parameter("ck", [128, T], F32, isOutput=False)
    sk = nc.declare_dram_parameter("sk", [128, T], F32, isOutput=False)
    outT = nc.declare_dram_parameter("outT", [256, T], F32, isOutput=True)

    rg = [list(range(NCORES))]

    with tile.TileContext(nc) as tc, ExitStack() as ctx:
        const = ctx.enter_context(tc.tile_pool(name="const", bufs=1))
        wpool = ctx.enter_context(tc.tile_pool(name="wpool", bufs=1))
        act = ctx.enter_context(tc.tile_pool(name="act", bufs=1))
        dram = ctx.enter_context(tc.tile_pool(name="dram", bufs=1, space="DRAM"))

        # ---- constants ----
        ones128 = const.tile([128, 128], F32)
        nc.vector.memset(ones128[:], 1.0)
        ident = const.tile([128, 128], F32)
        nc.gpsimd.affine_select(
            ident[:], ones128[:], pattern=[[-1, 128]], base=0,
            channel_multiplier=1, compare_op=mybir.AluOpType.is_equal, fill=0.0,
        )
        ones_col = const.tile([128, 1], F32R)
        nc.vector.tensor_copy(ones_col[:], ones128[:, 0:1])
        ones_row = const.tile([1, 128], F32R)
        nc.vector.tensor_copy(ones_row[:], ones128[0:1, :])
        eps_col = const.tile([128, 1], F32)
        nc.vector.memset(eps_col[:], EPS)

        # ---- resident weights / tables ----
        wq_sb = wpool.tile([128, HPC * NK * 128], BF16)
        nc.sync.dma_start(wq_sb[:], wq[:, :])
        wk_sb = wpool.tile([128, NK * 128], BF16)
        nc.sync.dma_start(wk_sb[:], wk[:, :])
        wv_sb = wpool.tile([128, NK * 128], BF16)
        nc.sync.dma_start(wv_sb[:], wv[:, :])
        wo_sb = wpool.tile([128, 64 * 128], BF16)
        nc.sync.dma_start(wo_sb[:], wo[:, :])
        cq_sb = wpool.tile([128, T], F32)
        nc.sync.dma_start(cq_sb[:], cq[:, :])
        sq_sb = wpool.tile([128, T], F32)
        nc.sync.dma_start(sq_sb[:], sq[:, :])
        ck_sb = wpool.tile([128, T], F32)
        nc.sync.dma_start(ck_sb[:], ck[:, :])
        sk_sb = wpool.tile([128, T], F32)
        nc.sync.dma_start(sk_sb[:], sk[:, :])

        # ---- persistent activations ----
        qT = [act.tile([128, T], F32R, name=f"qT{h}") for h in range(HPC)]
        kT = act.tile([128, T], F32R)
        vN = act.tile([128, NS * 128], F32R)  # natural [S,D] as 16 s-tiles
        yT = [act.tile([128, T], BF16, name=f"yT{h}") for h in range(HPC)]

        # ================= Phase A: build x^T slice, AllGather =================
        xt_loc = [act.tile([128, T], BF16, name=f"xtloc{i}") for i in range(2)]
        with tc.tile_pool(name="pa_sb", bufs=3) as pa_sb, \
             tc.tile_pool(name="pa_ps", bufs=4, space="PSUM") as pa_ps:
            for tt in range(16):
                xtile = pa_sb.tile([128, 256], F32, tag="xtile")
                nc.sync.dma_start(xtile[:], xs[tt * 128:(tt + 1) * 128, :])
                for cc in range(2):
                    pt = pa_ps.tile([128, 128], F32, tag="pt")
                    nc.tensor.transpose(pt[:], xtile[:, cc * 128:(cc + 1) * 128], ident[:])
                    nc.scalar.copy(xt_loc[cc][:, tt * 128:(tt + 1) * 128], pt[:])

        xt_in = dram.tile([256, T], BF16)
        for cc in range(2):
            nc.sync.dma_start(xt_in[cc * 128:(cc + 1) * 128, :], xt_loc[cc][:])
        xt_all = dram.tile([C, T], BF16, addr_space="Shared")
        nc.gpsimd.collective_compute(
            "AllGather", mybir.AluOpType.bypass, replica_groups=rg,
            ins=[xt_in[:].opt()], outs=[xt_all[:].opt()],
        )

        # ============ Phase B+C: QKV projections + RMSNorm + RoPE ============
        with tc.tile_pool(name="pb_sb", bufs=3) as pb_sb, \
             tc.tile_pool(name="pc_sb", bufs=2) as pc_sb, \
             tc.tile_pool(name="pb_ps", bufs=1, space="PSUM") as pb_ps, \
             tc.tile_pool(name="pc_ps", bufs=2, space="PSUM") as pc_ps:

            def norm_rope(ps, cos_sb, sin_sb, j, dest):
                js = slice(j * 512, (j + 1) * 512)
                raw = pc_sb.tile([128, 512], F32, tag="raw")
                nc.scalar.copy(raw[:], ps[:])
                sqr = pc_sb.tile([128, 512], F32R, tag="sqr")
                nc.vector.tensor_mul(sqr[:], raw[:], raw[:])
                ssq = pc_ps.tile([128, 512], F32, tag="cps")
                nc.tensor.matmul(
                    ssq[0:1, :], ones_col[:], sqr[:]
                )
                rms = pc_sb.tile([1, 512], F32, tag="rms")
                nc.scalar.activation(
                    rms[:], ssq[0:1, :], mybir.ActivationFunctionType.Sqrt,
                    scale=1.0 / 128.0, bias=eps_col[0:1, :],
                )
                rinv = pc_sb.tile([1, 512], F32R, tag="rinv")
                with nc.allow_low_precision(reason="f32r output feeds PE broadcast"):
                    nc.vector.reciprocal(rinv[:], rms[:])
                rb = pc_ps.tile([128, 512], F32, tag="cps")
                nc.tensor.matmul(
                    rb[:], ones_row[:], rinv[:]
                )
                qn = pc_sb.tile([128, 512], F32, tag="qn")
                nc.vector.tensor_mul(qn[:], raw[:], rb[:])
                qs = pc_sb.tile([128, 512], F32, tag="qs")
                nc.vector.stream_shuffle(qs[:], qn[:], mask=SWAP_MASK)
                t1 = pc_sb.tile([128, 512], F32, tag="t1")
                nc.vector.tensor_mul(t1[:], qn[:], cos_sb[:, js])
                t2 = pc_sb.tile([128, 512], F32, tag="t2")
                nc.vector.tensor_mul(t2[:], qs[:], sin_sb[:, js])
                nc.vector.tensor_add(dest[:, js], t1[:], t2[:])

            for j in range(NT):
                js = slice(j * 512, (j + 1) * 512)
                ps_q = [
                    pb_ps.tile([128, 512], F32, tag=f"psq{h}", name=f"psq{h}")
                    for h in range(HPC)
                ]
                ps_k = pb_ps.tile([128, 512], F32, tag="psk")
                ps_v = pb_ps.tile([128, 512], F32, tag="psv")
                for k in range(NK):
                    xk = pb_sb.tile([128, 512], BF16, tag="xk")
                    nc.sync.dma_start(
                        xk[:], xt_all[k * 128:(k + 1) * 128, js]
                    )
                    st = dict(start=(k == 0), stop=(k == NK - 1))
                    for h in range(HPC):
                        nc.tensor.matmul(
                            ps_q[h][:],
                            wq_sb[:, (h * NK + k) * 128:(h * NK + k + 1) * 128],
                            xk[:], **st,
                        )
                    nc.tensor.matmul(
                        ps_k[:], wk_sb[:, k * 128:(k + 1) * 128], xk[:], **st
                    )
                    nc.tensor.matmul(
                        ps_v[:], wv_sb[:, k * 128:(k + 1) * 128], xk[:], **st
                    )
                for h in range(HPC):
                    norm_rope(ps_q[h], cq_sb, sq_sb, j, qT[h])
                norm_rope(ps_k, ck_sb, sk_sb, j, kT)
                # v: no norm/rope; transpose [D,T]-chunk into natural [S,D] tiles
                vt = pc_sb.tile([128, 512], F32, tag="vt")
                nc.scalar.copy(vt[:], ps_v[:])
                for u in range(4):
                    s_tile = j * 4 + u
                    pvt = pc_ps.tile([128, 512], F32, tag="cps")
                    nc.tensor.transpose(
                        pvt[:, 0:128], vt[:, u * 128:(u + 1) * 128], ident[:]
                    )
                    nc.scalar.copy(
                        vN[:, s_tile * 128:(s_tile + 1) * 128], pvt[:, 0:128]
                    )

        # ===================== Phase D: attention =====================
        with tc.tile_pool(name="pd_sb", bufs=3) as pd_sb, \
             tc.tile_pool(name="pd_ps", bufs=1, space="PSUM") as pd_ps, \
             tc.tile_pool(name="ps_ps", bufs=2, space="PSUM") as ps_ps:
            for h in range(HPC):
                for j in range(NT):
                    js = slice(j * 512, (j + 1) * 512)
                    nblk = 4 * j + 4
                    ps_y = pd_ps.tile([128, 512], F32, tag="psy")
                    ps_den = pd_ps.tile([128, 512], F32, tag="psden")
                    for i in range(nblk):
                        ps_s = ps_ps.tile([128, 512], F32, tag="pss")
                        nc.tensor.matmul(
                            ps_s[:],
                            kT[:, i * 128:(i + 1) * 128],
                            qT[h][:, js],
                        )
                        et = pd_sb.tile([128, 512], F32R, tag="et")
                        nc.scalar.activation(
                            et[:], ps_s[:], mybir.ActivationFunctionType.Exp,
                            scale=float(SCALE),
                        )
                        if i >= 4 * j:  # diagonal block: causal mask
                            etm = pd_sb.tile([128, 512], F32R, tag="etm")
                            nc.gpsimd.affine_select(
                                etm[:], et[:], pattern=[[1, 512]],
                                base=512 * j - 128 * i, channel_multiplier=-1,
                                compare_op=mybir.AluOpType.is_ge, fill=0.0,
                            )
                            et = etm
                        st = dict(start=(i == 0), stop=(i == nblk - 1))
                        nc.tensor.matmul(
                            ps_y[:],
                            vN[:, i * 128:(i + 1) * 128],
                            et[:], **st,
                        )
                        nc.tensor.matmul(
                            ps_den[0:1, :], ones_col[:],
                            et[:], **st,
                        )
                    rd = pd_sb.tile([1, 512], F32R, tag="rd")
                    with nc.allow_low_precision(reason="f32r output feeds PE broadcast"):
                        nc.vector.reciprocal(rd[:], ps_den[0:1, :])
                    ps_rb = pd_ps.tile([128, 512], F32, tag="psrb")
                    nc.tensor.matmul(
                        ps_rb[:], ones_row[:], rd[:]
                    )
                    ytmp = pd_sb.tile([128, 512], F32, tag="ytmp")
                    nc.scalar.copy(ytmp[:], ps_y[:])
                    nc.vector.tensor_mul(yT[h][:, js], ytmp[:], ps_rb[:])

        # ================= Phase E: AllGather y^T =================
        y_in = dram.tile([HPC * 128, T], BF16)
        for h in range(HPC):
            nc.sync.dma_start(y_in[h * 128:(h + 1) * 128, :], yT[h][:])
        yt_all = dram.tile([N_HEAD * 128, T], BF16, addr_space="Shared")
        nc.gpsimd.collective_compute(
            "AllGather", mybir.AluOpType.bypass, replica_groups=rg,
            ins=[y_in[:].opt()], outs=[yt_all[:].opt()],
        )

        # ================= Phase F: o_proj (column shard) =================
        with tc.tile_pool(name="pf_sb", bufs=3) as pf_sb, \
             tc.tile_pool(name="pf_ps", bufs=1, space="PSUM") as pf_ps:
            ps_o = [
                [
                    pf_ps.tile([128, 512], F32, tag=f"pso{m}{j}", name=f"pso{m}{j}")
                    for j in range(NT)
                ]
                for m in range(2)
            ]
            for k in range(32):
                yk = pf_sb.tile([128, T], BF16, tag="yk")
                nc.sync.dma_start(yk[:], yt_all[k * 128:(k + 1) * 128, :])
                st = dict(start=(k == 0), stop=(k == 31))
                for m in range(2):
                    lh = wo_sb[:, (k * 2 + m) * 128:(k * 2 + m + 1) * 128]
                    for j in range(NT):
                        nc.tensor.matmul(
                            ps_o[m][j][:], lh, yk[:, j * 512:(j + 1) * 512], **st
                        )
            for m in range(2):
                for j in range(NT):
                    ot = pf_sb.tile([128, 512], F32, tag="ot")
                    nc.scalar.copy(ot[:], ps_o[m][j][:])
                    nc.sync.dma_start(
                        outT[m * 128:(m + 1) * 128, j * 512:(j + 1) * 512], ot[:]
                    )

    split_multiwaits(nc)
    return nc


_NC_CACHE = None


def _get_program():
    global _NC_CACHE
    if _NC_CACHE is None:
        _NC_CACHE = build_program()
    return _NC_CACHE


def make_inputs(x, input_pos, Wq, Wk, Wv, Wo, q_norm_w, k_norm_w):
    """Host-side sharding / layout prep. Returns per-core input maps."""
    bf16 = _bf16()
    x2d = np.asarray(x, np.float32).reshape(T, C)
    Wq = np.asarray(Wq, np.float32)
    Wk = np.asarray(Wk, np.float32)
    Wv = np.asarray(Wv, np.float32)
    Wo = np.asarray(Wo, np.float32)
    q_norm_w = np.asarray(q_norm_w, np.float32)
    k_norm_w = np.asarray(k_norm_w, np.float32)
    pos = np.asarray(input_pos, np.float32)

    # interleaved head-dim permutation: [0, 64, 1, 65, ...]
    perm = np.empty(128, np.int64)
    perm[0::2] = np.arange(64)
    perm[1::2] = np.arange(64) + 64

    # rope tables in interleaved layout, norm weights folded in
    inv_freq = (THETA ** (-(np.arange(0, D, 2, dtype=np.float32)) / D)).astype(
        np.float32
    )
    fr = pos[:, None] * inv_freq[None, :]  # [T, 64]
    cos = np.cos(fr).astype(np.float32).T  # [64, T]
    sin = np.sin(fr).astype(np.float32).T
    cos_il = np.empty((128, T), np.float32)
    cos_il[0::2] = cos
    cos_il[1::2] = cos
    sin_eff = np.empty((128, T), np.float32)
    sin_eff[0::2] = -sin
    sin_eff[1::2] = sin

    def tables(w):
        w_il = w[perm]
        w_swap = w_il[[i ^ 1 for i in range(128)]]
        return (
            np.ascontiguousarray(cos_il * w_il[:, None]),
            np.ascontiguousarray(sin_eff * w_swap[:, None]),
        )

    cq_t, sq_t = tables(q_norm_w)
    ck_t, sk_t = tables(k_norm_w)

    Wq4 = Wq.reshape(N_HEAD, D, C)
    Wk4 = Wk.reshape(N_KV, D, C)
    Wv4 = Wv.reshape(N_KV, D, C)

    in_maps = []
    for c in range(NCORES):
        g = c // 2
        Wc = Wq4[HPC * c:HPC * (c + 1)][:, perm, :]  # [4, 128, C]
        wq_host = np.ascontiguousarray(
            Wc.reshape(HPC, 128, NK, 128).transpose(3, 0, 2, 1).reshape(128, -1)
        ).astype(bf16)
        wk_host = np.ascontiguousarray(
            Wk4[g][perm].reshape(128, NK, 128).transpose(2, 1, 0).reshape(128, -1)
        ).astype(bf16)
        wv_host = np.ascontiguousarray(
            Wv4[g].reshape(128, NK, 128).transpose(2, 1, 0).reshape(128, -1)
        ).astype(bf16)
        WoC = Wo[256 * c:256 * (c + 1), :]  # [256, 4096]
        wo_host = np.ascontiguousarray(
            WoC.reshape(2, 128, 32, 128).transpose(3, 2, 0, 1).reshape(128, -1)
        ).astype(bf16)
        xs_host = np.ascontiguousarray(x2d[:, 256 * c:256 * (c + 1)])
        in_maps.append(
            {
                "xs": xs_host,
                "wq": wq_host,
                "wk": wk_host,
                "wv": wv_host,
                "wo": wo_host,
                "cq": cq_t,
                "sq": sq_t,
                "ck": ck_t,
                "sk": sk_t,
            }
        )
    return in_maps


def kernel(x, input_pos, Wq, Wk, Wv, Wo, q_norm_w, k_norm_w):
    nc = _get_program()
    in_maps = make_inputs(x, input_pos, Wq, Wk, Wv, Wo, q_norm_w, k_norm_w)
    res = run_bass_kernel_spmd(nc, in_maps, list(range(NCORES)))
    out = np.empty((1, T, C), np.float32)
    for c in range(NCORES):
        out[0][:, 256 * c:256 * (c + 1)] = res.results[c]["outT"].T
    return out


# revision 13
# speedup vs baseline: 156.8191x; 156.8191x over previous
"""Trainium2 Bass kernel for causal self-attention (GQA, RoPE, q/k-RMSNorm).

Sharding: tensor-parallel over heads across 8 cores.
  - core c owns q-heads [4c, 4c+4) and kv-head c//2 (each kv head serves 8 q heads)
  - x^T is built locally on each core via DMA-transpose (bf16) and kept in SBUF
  - attention is computed transposed (E^T = exp(K·Q^T)) so V in natural [S,D]
    layout is the matmul lhsT and y^T comes out in [D,T] layout directly
  - y^T is AllGathered per head (4 collectives overlapped with attention);
    o_proj is column-sharded: core c computes Wo[256c:256c+256,:] @ y^T_full
  - head-dim rows of q/k are interleaved (d -> [0,64,1,65,...]) so the RoPE
    rotate-half becomes an adjacent-pair partition swap (one stream_shuffle)
  - rmsnorm scale and the norm weight are applied in one shot: the PE
    broadcast matmul computes w[p] * rinv[t] (lhsT = w row, rhs = 1/rms row)

Matmul dtypes: QKV + o_proj in bf16 (fp32 PSUM accum), attention in float32r.
"""

import sys

sys.path.insert(0, "/opt/trn_rl_repo")

from contextlib import ExitStack

import numpy as np

import bass_rust
import concourse.bass as bass
import concourse.mybir as mybir
from concourse import tile

F32 = mybir.dt.float32
F32R = mybir.dt.float32r
BF16 = mybir.dt.bfloat16

N_HEAD = 32
N_KV = 4
D = 128
C = 2048
T = 2048
NCORES = 8
HPC = N_HEAD // NCORES  # q heads per core = 4
THETA = 1000000.0
EPS = 1e-6
SCALE = 1.0 / np.sqrt(128.0)

NT = T // 512  # 4 T-chunks of 512
NK = C // 128  # 16 contraction tiles for qkv
NS = T // 128  # 16 S-blocks of 128

# stream_shuffle swaps within each 32-partition quadrant; adjacent-pair swap
SWAP_MASK = [i ^ 1 for i in range(32)]

_BF16_NP = None


def _bf16():
    global _BF16_NP
    if _BF16_NP is None:
        import ml_dtypes

        _BF16_NP = np.dtype(ml_dtypes.bfloat16)
    return _BF16_NP


def split_multiwaits(nc):
    """The walrus build in this container supports one sync-wait per
    instruction; hoist extra waits onto NOPs inserted before the offender."""
    ctr = 0
    for f in nc.m.functions:
        for bb in f.blocks:
            new_insts = []
            changed = False
            for inst in bb.instructions:
                si = inst.sync_info
                if si is not None and si.on_wait and len(si.on_wait) > 1:
                    waits = list(si.on_wait)
                    for w in waits[:-1]:
                        ctr += 1
                        nop = bass_rust.InstNoOp(name=f"splitw-{ctr}", ins=[], outs=[])
                        nop.engine = inst.engine
                        nop.sync_info = bass_rust.SyncInfo(on_wait=[w], on_update=[])
                        new_insts.append(nop)
                    inst.sync_info = bass_rust.SyncInfo(
                        on_wait=[waits[-1]], on_update=list(si.on_update or [])
                    )
                    changed = True
                new_insts.append(inst)
            if changed:
                bb.instructions = new_insts


def build_program(bench_reps=0):
    nc = bass.Bass("TRN2", target_bir_lowering=False, debug=False, num_devices=NCORES)

    xb = nc.declare_dram_parameter("xb", [T, C], BF16, isOutput=False)
    wq = nc.declare_dram_parameter("wq", [128, HPC * NK * 128], BF16, isOutput=False)
    wk = nc.declare_dram_parameter("wk", [128, NK * 128], BF16, isOutput=False)
    wv = nc.declare_dram_parameter("wv", [128, NK * 128], BF16, isOutput=False)
    wo = nc.declare_dram_parameter("wo", [128, 32 * 256], BF16, isOutput=False)
    cost = nc.declare_dram_parameter("cost", [128, T], F32, isOutput=False)
    sint = nc.declare_dram_parameter("sint", [128, T], F32, isOutput=False)
    wqn = nc.declare_dram_parameter("wqn", [1, 128], F32, isOutput=False)
    wkn = nc.declare_dram_parameter("wkn", [1, 128], F32, isOutput=False)
    outT = nc.declare_dram_parameter("outT", [256, T], F32, isOutput=True)

    rg = [list(range(NCORES))]
    collectives = bench_reps == 0

    with tile.TileContext(nc) as tc, ExitStack() as ctx:
        const = ctx.enter_context(tc.tile_pool(name="const", bufs=1))
        wpool = ctx.enter_context(tc.tile_pool(name="wpool", bufs=1))
        act = ctx.enter_context(tc.tile_pool(name="act", bufs=1))
        dram = ctx.enter_context(tc.tile_pool(name="dram", bufs=1, space="DRAM"))

        # ---- constants ----
        ones128 = const.tile([128, 128], F32)
        nc.vector.memset(ones128[:], 1.0)
        ident = const.tile([128, 128], F32)
        nc.gpsimd.affine_select(
            ident[:], ones128[:], pattern=[[-1, 128]], base=0,
            channel_multiplier=1, compare_op=mybir.AluOpType.is_equal, fill=0.0,
        )
        ones_col = const.tile([128, 1], F32R)
        nc.vector.tensor_copy(ones_col[:], ones128[:, 0:1])
        ones_row = const.tile([1, 128], F32R)
        nc.vector.tensor_copy(ones_row[:], ones128[0:1, :])
        eps_col = const.tile([128, 1], F32)
        nc.vector.memset(eps_col[:], EPS)

        # ---- resident weights / tables ----
        wq_sb = wpool.tile([128, HPC * NK * 128], BF16)
        nc.sync.dma_start(wq_sb[:], wq[:, :])
        wk_sb = wpool.tile([128, NK * 128], BF16)
        nc.sync.dma_start(wk_sb[:], wk[:, :])
        wv_sb = wpool.tile([128, NK * 128], BF16)
        nc.sync.dma_start(wv_sb[:], wv[:, :])
        cos_sb = wpool.tile([128, T], F32)
        nc.sync.dma_start(cos_sb[:], cost[:, :])
        sin_sb = wpool.tile([128, T], F32)
        nc.sync.dma_start(sin_sb[:], sint[:, :])
        wqn_f = wpool.tile([1, 128], F32)
        nc.sync.dma_start(wqn_f[:], wqn[:, :])
        wkn_f = wpool.tile([1, 128], F32)
        nc.sync.dma_start(wkn_f[:], wkn[:, :])
        wqn_sb = wpool.tile([1, 128], F32R)
        nc.vector.tensor_copy(wqn_sb[:], wqn_f[:])
        wkn_sb = wpool.tile([1, 128], F32R)
        nc.vector.tensor_copy(wkn_sb[:], wkn_f[:])

        # ---- persistent activations ----
        qT = [act.tile([128, T], F32R, name=f"qT{h}") for h in range(HPC)]
        kT = act.tile([128, T], F32R)
        vN = act.tile([128, NS * 128], F32R)  # natural [S,D] as 16 s-tiles
        yT = [act.tile([128, T], BF16, name=f"yT{h}") for h in range(HPC)]

        # DRAM bounce + collective buffers
        y_in = [dram.tile([128, T], BF16, name=f"yin{h}") for h in range(HPC)]
        yt_all = [
            dram.tile(
                [NCORES * 128, T], BF16, name=f"ytall{h}",
                addr_space="Shared" if collectives else "Local",
            )
            for h in range(HPC)
        ]

        def body():
            # ===== Phase A: x^T via DMA transpose (bf16), kept in SBUF =====
            with tc.tile_pool(name="xtp", bufs=1) as xtp:
                xT = [xtp.tile([128, T], BF16, name=f"xT{k}") for k in range(NK)]
                for k in range(NK):
                    nc.sync.dma_start_transpose(
                        xT[k][:], xb[:, k * 128:(k + 1) * 128]
                    )

                # ===== Phase B+C: QKV + RMSNorm + RoPE =====
                with tc.tile_pool(name="pc_sb", bufs=2) as pc_sb, \
                     tc.tile_pool(name="pb_ps", bufs=1, space="PSUM") as pb_ps, \
                     tc.tile_pool(name="pc_ps", bufs=2, space="PSUM") as pc_ps:

                    def norm_rope(ps, w_row, j, dest):
                        js = slice(j * 512, (j + 1) * 512)
                        raw = pc_sb.tile([128, 512], F32, tag="raw")
                        nc.vector.tensor_copy(raw[:], ps[:])
                        sqr = pc_sb.tile([128, 512], F32R, tag="sqr")
                        nc.vector.tensor_mul(sqr[:], raw[:], raw[:])
                        ssq = pc_ps.tile([128, 512], F32, tag="cps")
                        nc.tensor.matmul(ssq[0:1, :], ones_col[:], sqr[:])
                        rms = pc_sb.tile([1, 512], F32, tag="rms")
                        nc.scalar.activation(
                            rms[:], ssq[0:1, :], mybir.ActivationFunctionType.Sqrt,
                            scale=1.0 / 128.0, bias=eps_col[0:1, :],
                        )
                        rinv = pc_sb.tile([1, 512], F32R, tag="rinv")
                        with nc.allow_low_precision(reason="feeds PE broadcast"):
                            nc.vector.reciprocal(rinv[:], rms[:])
                        # rb[p,t] = w[p] * rinv[t]  (rank-1 PE broadcast)
                        rb = pc_ps.tile([128, 512], F32, tag="cps")
                        nc.tensor.matmul(rb[:], w_row[:], rinv[:])
                        qn = pc_sb.tile([128, 512], F32, tag="qn")
                        nc.vector.tensor_mul(qn[:], raw[:], rb[:])
                        qs = pc_sb.tile([128, 512], F32, tag="qs")
                        nc.vector.stream_shuffle(qs[:], qn[:], mask=SWAP_MASK)
                        t1 = pc_sb.tile([128, 512], F32, tag="t1")
                        nc.vector.tensor_mul(t1[:], qn[:], cos_sb[:, js])
                        t2 = pc_sb.tile([128, 512], F32, tag="t2")
                        nc.vector.tensor_mul(t2[:], qs[:], sin_sb[:, js])
                        nc.vector.tensor_add(dest[:, js], t1[:], t2[:])

                    for j in range(NT):
                        js = slice(j * 512, (j + 1) * 512)
                        ps_q = [
                            pb_ps.tile([128, 512], F32, tag=f"psq{h}", name=f"psq{h}")
                            for h in range(HPC)
                        ]
                        ps_k = pb_ps.tile([128, 512], F32, tag="psk")
                        ps_v = pb_ps.tile([128, 512], F32, tag="psv")
                        for k in range(NK):
                            st = dict(start=(k == 0), stop=(k == NK - 1))
                            rhs = xT[k][:, js]
                            for h in range(HPC):
                                nc.tensor.matmul(
                                    ps_q[h][:],
                                    wq_sb[:, (h * NK + k) * 128:(h * NK + k + 1) * 128],
                                    rhs, **st,
                                )
                            nc.tensor.matmul(
                                ps_k[:], wk_sb[:, k * 128:(k + 1) * 128], rhs, **st
                            )
                            nc.tensor.matmul(
                                ps_v[:], wv_sb[:, k * 128:(k + 1) * 128], rhs, **st
                            )
                        for h in range(HPC):
                            norm_rope(ps_q[h], wqn_sb, j, qT[h])
                        norm_rope(ps_k, wkn_sb, j, kT)
                        # v: transpose [D,T]-chunk into natural [S,D] tiles
                        vt = pc_sb.tile([128, 512], F32, tag="vt")
                        nc.vector.tensor_copy(vt[:], ps_v[:])
                        for u in range(4):
                            s_tile = j * 4 + u
                            pvt = pc_ps.tile([128, 512], F32, tag="cps")
                            nc.tensor.transpose(
                                pvt[:, 0:128], vt[:, u * 128:(u + 1) * 128], ident[:]
                            )
                            nc.vector.tensor_copy(
                                vN[:, s_tile * 128:(s_tile + 1) * 128], pvt[:, 0:128]
                            )

            # ===== Phase D: attention (+ per-head y AllGather) =====
            with tc.tile_pool(name="pd_sb", bufs=3) as pd_sb, \
                 tc.tile_pool(name="pd_ps", bufs=1, space="PSUM") as pd_ps, \
                 tc.tile_pool(name="ps_ps", bufs=2, space="PSUM") as ps_ps:
                for h in range(HPC):
                    for j in range(NT):
                        js = slice(j * 512, (j + 1) * 512)
                        nblk = 4 * j + 4
                        ps_y = pd_ps.tile([128, 512], F32, tag="psy")
                        ps_den = pd_ps.tile([128, 512], F32, tag="psden")
                        for i in range(nblk):
                            ps_s = ps_ps.tile([128, 512], F32, tag="pss")
                            nc.tensor.matmul(
                                ps_s[:], kT[:, i * 128:(i + 1) * 128], qT[h][:, js]
                            )
                            et = pd_sb.tile([128, 512], F32R, tag="et")
                            nc.scalar.activation(
                                et[:], ps_s[:], mybir.ActivationFunctionType.Exp,
                                scale=float(SCALE),
                            )
                            if i >= 4 * j:  # diagonal block: causal mask
                                etm = pd_sb.tile([128, 512], F32R, tag="etm")
                                nc.gpsimd.affine_select(
                                    etm[:], et[:], pattern=[[1, 512]],
                                    base=512 * j - 128 * i, channel_multiplier=-1,
                                    compare_op=mybir.AluOpType.is_ge, fill=0.0,
                                )
                                et = etm
                            st = dict(start=(i == 0), stop=(i == nblk - 1))
                            nc.tensor.matmul(
                                ps_y[:], vN[:, i * 128:(i + 1) * 128], et[:], **st
                            )
                            nc.tensor.matmul(
                                ps_den[0:1, :], ones_col[:], et[:], **st
                            )
                        rd = pd_sb.tile([1, 512], F32R, tag="rd")
                        with nc.allow_low_precision(reason="feeds PE broadcast"):
                            nc.vector.reciprocal(rd[:], ps_den[0:1, :])
                        ps_rb = pd_ps.tile([128, 512], F32, tag="psrb")
                        nc.tensor.matmul(ps_rb[:], ones_row[:], rd[:])
                        ytmp = pd_sb.tile([128, 512], F32, tag="ytmp")
                        nc.vector.tensor_copy(ytmp[:], ps_y[:])
                        nc.vector.tensor_mul(yT[h][:, js], ytmp[:], ps_rb[:])
                    # gather this head's y^T across cores
                    nc.sync.dma_start(y_in[h][:, :], yT[h][:])
                    if collectives:
                        nc.gpsimd.collective_compute(
                            "AllGather", mybir.AluOpType.bypass, replica_groups=rg,
                            ins=[y_in[h][:].opt()], outs=[yt_all[h][:].opt()],
                        )

            # ===== Phase F: o_proj (column shard) =====
            with tc.tile_pool(name="pf_sb", bufs=3) as pf_sb, \
                 tc.tile_pool(name="pf_ps", bufs=1, space="PSUM") as pf_ps:
                ps_o = [
                    [
                        pf_ps.tile([128, 512], F32, tag=f"pso{m}{j}", name=f"pso{m}{j}")
                        for j in range(NT)
                    ]
                    for m in range(2)
                ]
                for h in range(HPC):
                    for cp in range(NCORES):
                        k = 4 * cp + h  # global head index = wo k-tile index
                        yk = pf_sb.tile([128, T], BF16, tag="yk")
                        nc.sync.dma_start(
                            yk[:], yt_all[h][cp * 128:(cp + 1) * 128, :]
                        )
                        wo_t = pf_sb.tile([128, 256], BF16, tag="wot")
                        nc.sync.dma_start(wo_t[:], wo[:, k * 256:(k + 1) * 256])
                        st = dict(
                            start=(h == 0 and cp == 0), stop=(h == HPC - 1 and cp == 7)
                        )
                        for m in range(2):
                            lh = wo_t[:, m * 128:(m + 1) * 128]
                            for j in range(NT):
                                nc.tensor.matmul(
                                    ps_o[m][j][:], lh, yk[:, j * 512:(j + 1) * 512],
                                    **st,
                                )
                for m in range(2):
                    for j in range(NT):
                        ot = pf_sb.tile([128, 512], F32, tag="ot")
                        nc.vector.tensor_copy(ot[:], ps_o[m][j][:])
                        nc.sync.dma_start(
                            outT[m * 128:(m + 1) * 128, j * 512:(j + 1) * 512], ot[:]
                        )

        if bench_reps:
            with tc.For_i(0, bench_reps, 1):
                body()
        else:
            body()

    split_multiwaits(nc)
    return nc


# ---------------------------------------------------------------------------
# host side
# ---------------------------------------------------------------------------

_RUNNER_CACHE = None


def _make_runner(nc, n_cores=NCORES):
    """Build the sharded jit once; returns run(in_maps) -> list of out dicts."""
    import jax
    from jax.sharding import Mesh, NamedSharding, PartitionSpec
    from jax.experimental.shard_map import shard_map
    from concourse import bass2jax
    from concourse.bass2jax import _bass_exec_p, partition_id_tensor

    bass2jax.install_neuronx_cc_hook()

    partition_name = nc.partition_id_tensor.name if nc.partition_id_tensor else None
    in_names, out_names, out_avals, zero_outs = [], [], [], []
    for alloc in nc.m.functions[0].allocations:
        if not isinstance(alloc, mybir.MemoryLocationSet):
            continue
        name = alloc.memorylocations[0].name
        if alloc.kind == "ExternalInput":
            if name != partition_name:
                in_names.append(name)
        elif alloc.kind == "ExternalOutput":
            out_names.append(name)
            shape = tuple(alloc.tensor_shape)
            dtype = mybir.dt.np(alloc.dtype)
            out_avals.append(jax.core.ShapedArray(shape, dtype))
            zero_outs.append(np.zeros(shape, dtype))
    n_params = len(in_names)
    n_outs = len(out_avals)
    all_in_names = list(in_names) + list(out_names)
    if partition_name is not None:
        all_in_names.append(partition_name)
    donate = tuple(range(n_params, n_params + n_outs))

    def _body(*args):
        operands = list(args)
        if partition_name is not None:
            operands.append(partition_id_tensor())
        outs = _bass_exec_p.bind(
            *operands,
            out_avals=tuple(out_avals),
            in_names=tuple(all_in_names),
            out_names=tuple(out_names),
            lowering_input_output_aliases=(),
            sim_require_finite=True,
            sim_require_nnan=True,
            nc=nc,
        )
        return tuple(outs)

    devices = jax.devices()[:n_cores]
    mesh = Mesh(np.asarray(devices), ("core",))
    sharded = jax.jit(
        shard_map(
            _body, mesh=mesh,
            in_specs=(PartitionSpec("core"),) * (n_params + n_outs),
            out_specs=(PartitionSpec("core"),) * n_outs,
            check_rep=False,
        ),
        donate_argnums=donate,
        keep_unused=True,
    )
    shard = NamedSharding(mesh, PartitionSpec("core"))
    zshapes = [((n_cores * z.shape[0],) + z.shape[1:], z.dtype) for z in zero_outs]

    def run(in_maps):
        concat_in = [
            jax.device_put(
                np.concatenate(
                    [np.asarray(in_maps[c][n]) for c in range(n_cores)], axis=0
                ),
                shard,
            )
            for n in in_names
        ]
        zs = [jax.device_put(np.zeros(s, d), shard) for s, d in zshapes]
        outs = sharded(*concat_in, *zs)
        return [
            {
                name: np.asarray(outs[i]).reshape(n_cores, *out_avals[i].shape)[c]
                for i, name in enumerate(out_names)
            }
            for c in range(n_cores)
        ]

    return run


def _get_runner():
    global _RUNNER_CACHE
    if _RUNNER_CACHE is None:
        _RUNNER_CACHE = _make_runner(build_program())
    return _RUNNER_CACHE


def make_inputs(x, input_pos, Wq, Wk, Wv, Wo, q_norm_w, k_norm_w):
    """Host-side sharding / layout prep. Returns per-core input maps."""
    bf16 = _bf16()
    x2d = np.ascontiguousarray(np.asarray(x, np.float32).reshape(T, C)).astype(bf16)
    Wq = np.asarray(Wq, np.float32)
    Wk = np.asarray(Wk, np.float32)
    Wv = np.asarray(Wv, np.float32)
    Wo = np.asarray(Wo, np.float32)
    q_norm_w = np.asarray(q_norm_w, np.float32)
    k_norm_w = np.asarray(k_norm_w, np.float32)
    pos = np.asarray(input_pos, np.float32)

    # interleaved head-dim permutation: [0, 64, 1, 65, ...]
    perm = np.empty(128, np.int64)
    perm[0::2] = np.arange(64)
    perm[1::2] = np.arange(64) + 64

    # rope tables in interleaved layout (sign of the rotate-half folded in)
    inv_freq = (THETA ** (-(np.arange(0, D, 2, dtype=np.float32)) / D)).astype(
        np.float32
    )
    fr = pos[:, None] * inv_freq[None, :]  # [T, 64]
    cos = np.cos(fr).astype(np.float32).T  # [64, T]
    sin = np.sin(fr).astype(np.float32).T
    cos_il = np.empty((128, T), np.float32)
    cos_il[0::2] = cos
    cos_il[1::2] = cos
    sin_eff = np.empty((128, T), np.float32)
    sin_eff[0::2] = -sin
    sin_eff[1::2] = sin
    cos_il = np.ascontiguousarray(cos_il)
    sin_eff = np.ascontiguousarray(sin_eff)
    wqn_h = np.ascontiguousarray(q_norm_w[perm][None, :])
    wkn_h = np.ascontiguousarray(k_norm_w[perm][None, :])

    Wq4 = Wq.reshape(N_HEAD, D, C)
    Wk4 = Wk.reshape(N_KV, D, C)
    Wv4 = Wv.reshape(N_KV, D, C)

    in_maps = []
    for c in range(NCORES):
        g = c // 2
        Wc = Wq4[HPC * c:HPC * (c + 1)][:, perm, :]  # [4, 128, C]
        wq_host = np.ascontiguousarray(
            Wc.reshape(HPC, 128, NK, 128).transpose(3, 0, 2, 1).reshape(128, -1)
        ).astype(bf16)
        wk_host = np.ascontiguousarray(
            Wk4[g][perm].reshape(128, NK, 128).transpose(2, 1, 0).reshape(128, -1)
        ).astype(bf16)
        wv_host = np.ascontiguousarray(
            Wv4[g].reshape(128, NK, 128).transpose(2, 1, 0).reshape(128, -1)
        ).astype(bf16)
        WoC = Wo[256 * c:256 * (c + 1), :]  # [256, 4096]
        wo_host = np.ascontiguousarray(
            WoC.reshape(2, 128, 32, 128).transpose(3, 2, 0, 1).reshape(128, -1)
        ).astype(bf16)
        in_maps.append(
            {
                "xb": x2d,
                "wq": wq_host,
                "wk": wk_host,
                "wv": wv_host,
                "wo": wo_host,
                "cost": cos_il,
                "sint": sin_eff,
                "wqn": wqn_h,
                "wkn": wkn_h,
            }
        )
    return in_maps


def kernel(x, input_pos, Wq, Wk, Wv, Wo, q_norm_w, k_norm_w):
    run = _get_runner()
    in_maps = make_inputs(x, input_pos, Wq, Wk, Wv, Wo, q_norm_w, k_norm_w)
    results = run(in_maps)
    out = np.empty((1, T, C), np.float32)
    for c in range(NCORES):
        out[0][:, 256 * c:256 * (c + 1)] = results[c]["outT"].T
    return out
